# revision 1
# baseline (speedup 1.0000x reference)
"""GAT conv layer (B=2, N=4096, C=256, H=4, D=64) on TRN2 NeuronCores.

Execution-environment reality (measured): instructions dispatch at ~27us
each, serialized across ALL engines and (beyond 2 cores) across cores;
marginal per-element rates are also ~5-10x architectural spec.  The design
therefore minimizes TOTAL INSTRUCTION COUNT rather than classic per-engine
occupancy, and shards across only 2 cores (one batch per core) since extra
cores serialize anyway and would duplicate fixed work.

Per core (batch b): scores kept in [j = source node (partition), i = target
node (free)] layout so softmax needs no on-chip reduction: the attention
matmul's stationary operand is [Wh | 1] (fp16) whose ones column accumulates
the softmax denominator next to the numerator.

Score pipeline per [128, 8192] fp16 tile (DVE scalar_tensor_tensor):
  z = (srcB + tgt[j]) - m255      STT per 2048-col block (per-jc tgt scalar)
  l = max(0.2*z, z)               one STT  (= leaky_relu; masked entries
                                  land at 0.2*e - 51 -> exp ~ 1e-22)
  p = Exp(l) -> fp16              one ACT pass
  psum[65, 2048-block] += [Wh|1].T @ p     PE fp16, N=512 per matmul
Heads processed in pairs (psum: 4 banks per head-accumulator x 2).
Normalization: reciprocal of the denominator row, PE-transpose to [i, d]
blocks, fused scale+head-accumulate (0.25 head-mean baked into W on host).
"""

import numpy as np

B, N, C, H, D = 2, 4096, 256, 4, 64
NEG = 0.2
JC = N // 128        # 32 source chunks
W_I = 2048           # i-block width (psum: [65, 2048] = 4 banks per head)
G = 4                # j-chunks per score tile -> z tiles [128, G*W_I]
MASKV = 255.0
USE_FP8 = False       # fp8 DoubleRow halves matmul count but costs ~30x precision

_cached = {}


def _build(reps=1):
    import concourse.bacc as bacc
    import concourse.tile as tile
    from concourse import mybir
    from concourse.masks import make_identity

    f32 = mybir.dt.float32
    f16 = mybir.dt.float16
    u8 = mybir.dt.uint8
    Alu = mybir.AluOpType

    nc = bacc.Bacc(None, target_bir_lowering=False, name="gat2")

    # waug column layout per jc-projection: for h in 0..3: [0.25*W_h (64) | w_tgt_h]
    xT = nc.dram_tensor("xT", [2, 128, N], f32, kind="ExternalInput")
    waug = nc.dram_tensor("waug", [2, 128, H * 65], f32, kind="ExternalInput")
    wsb = nc.dram_tensor("wsb", [H, 2, 128, 128], f32, kind="ExternalInput")
    # mprep[ib, jcg, p, (jl, i)] with value 255*mask, uint8
    mprep = nc.dram_tensor("mprep", [2, 8, 128, G * W_I], u8, kind="ExternalInput")
    outd = nc.dram_tensor("out", [128, (N // 128) * D], f32, kind="ExternalOutput")

    def pipeline(tc, const, whaug, tgt16, srcB, ident, neg3):
        # ---------------- phase A: projection  [Wh|tgt] = x @ waug --------
        with tc.tile_pool(name="ld", bufs=1) as ld, \
             tc.tile_pool(name="psA", bufs=4, space="PSUM") as psA:
            xT_sb = ld.tile([128, 2 * N], f32)
            waug_sb = ld.tile([128, 2 * H * 65], f32)
            wsb_sb = ld.tile([128, H * 2 * 128], f32)
            for cc in range(2):
                nc.sync.dma_start(xT_sb[:, cc * N:(cc + 1) * N], xT[cc])
                nc.sync.dma_start(
                    waug_sb[:, cc * H * 65:(cc + 1) * H * 65], waug[cc])
                for h in range(H):
                    nc.sync.dma_start(
                        wsb_sb[:, (h * 2 + cc) * 128:(h * 2 + cc + 1) * 128],
                        wsb[h, cc])

            for jc in range(JC):
                psp = psA.tile([128, H * 65], f32, tag="psp")
                for cc in range(2):
                    nc.tensor.matmul(
                        psp,
                        xT_sb[:, cc * N + jc * 128: cc * N + (jc + 1) * 128],
                        waug_sb[:, cc * H * 65:(cc + 1) * H * 65],
                        start=(cc == 0), stop=(cc == 1))
                if USE_FP8:
                    # interleave jc-pairs for DoubleRow: whaug[p, h, jc//2, jc%2, m]
                    wview = whaug.rearrange(
                        "p (h jcp t m) -> p h (jcp t) m", h=H, t=2, m=80)
                    nc.scalar.copy(
                        wview[:, :, jc, 0:65],
                        psp.rearrange("p (h m) -> p h m", h=H))
                    nc.scalar.copy(
                        tgt16.rearrange(
                            "p (jc h) -> p jc h", h=H)[:, jc, :],
                        psp[:, 64::65])
                else:
                    nc.scalar.copy(
                        whaug[:, jc * H * 65:(jc + 1) * H * 65], psp)
            if USE_FP8:
                ones_cols = whaug.rearrange(
                    "p (g m) -> p g m", m=80)[:, :, 64:65]
                nc.vector.memset(ones_cols, 1.0)
            else:
                tgt_cols = whaug.rearrange(
                    "p (jch l) -> p jch l", l=65)[:, :, 64:65]
                nc.vector.tensor_copy(
                    tgt16.rearrange("p (jch one) -> p jch one", one=1), tgt_cols)
                nc.vector.memset(tgt_cols, 1.0)

            # -------------- phase B: srcB_h = broadcast(x @ wsrc_h) -------
            with tc.tile_pool(name="psB", bufs=1, space="PSUM") as psB:
                for h in range(H):
                    for half in range(2):
                        pss = psB.tile([128, N // 2], f32, tag="pss")
                        for iq in range(4):
                            for cc in range(2):
                                nc.tensor.matmul(
                                    pss[:, iq * 512:(iq + 1) * 512],
                                    wsb_sb[:, (h * 2 + cc) * 128:(h * 2 + cc + 1) * 128],
                                    xT_sb[:, cc * N + half * 2048 + iq * 512:
                                          cc * N + half * 2048 + (iq + 1) * 512],
                                    start=(cc == 0), stop=(cc == 1),
                                    skip_group_check=True)
                        nc.scalar.copy(
                            srcB[:, h * N + half * 2048: h * N + (half + 1) * 2048],
                            pss)

        # ---------------- phase C: scores + attention matmul --------------
        with tc.tile_pool(name="nd", bufs=1) as ndp:
          nd = [ndp.tile([65, N], f32, name=f"nd{h}", tag=f"nd{h}")
                for h in range(H)]
          with tc.tile_pool(name="mw", bufs=1) as mw, \
               tc.tile_pool(name="zw", bufs=1) as zw, \
               tc.tile_pool(name="lw", bufs=1) as lw, \
               tc.tile_pool(name="pw", bufs=1) as pw, \
               tc.tile_pool(name="psC", bufs=1, space="PSUM") as psC:
              for ib in range(2):
                  for hp in range(2):
                      acc = [psC.tile([65, W_I], f32, name=f"acc{a}", tag=f"acc{a}")
                             for a in range(2)]
                      for jcg in range(8):
                          m_t = mw.tile([128, G * W_I], u8, tag="m")
                          nc.sync.dma_start(m_t, mprep[ib, jcg])
                          for hh in range(2):
                              h = hp * 2 + hh
                              z_t = zw.tile([128, G * W_I], f16, tag="z")
                              for jl in range(G):
                                  jc = jcg * G + jl
                                  nc.vector.scalar_tensor_tensor(
                                      out=z_t[:, jl * W_I:(jl + 1) * W_I],
                                      in0=srcB[:, h * N + ib * W_I:
                                               h * N + ib * W_I + W_I],
                                      scalar=tgt16[:, jc * H + h: jc * H + h + 1],
                                      in1=m_t[:, jl * W_I:(jl + 1) * W_I],
                                      op0=Alu.add, op1=Alu.subtract)
                              l_t = lw.tile([128, G * W_I], f16, tag="l")
                              nc.vector.scalar_tensor_tensor(
                                  out=l_t, in0=z_t, scalar=NEG, in1=z_t,
                                  op0=Alu.mult, op1=Alu.max)
                              pdt = mybir.dt.float8e4 if USE_FP8 else f16
                              p_t = pw.tile([128, G * W_I], pdt, tag="p")
                              nc.scalar.activation(
                                  out=p_t, in_=l_t,
                                  func=mybir.ActivationFunctionType.Exp,
                                  bias=(neg3 if USE_FP8 else 0.0))
                              if USE_FP8:
                                  wv = whaug.rearrange(
                                      "p (hh2 jcp t m) -> p hh2 jcp t m",
                                      hh2=H, t=2, m=80)
                                  pv = p_t.rearrange(
                                      "p (jl i) -> p jl i", jl=G)
                                  for u in range(G // 2):
                                      jcp = jcg * (G // 2) + u
                                      for q in range(W_I // 512):
                                          nc.tensor.matmul(
                                              acc[hh][:, q * 512:(q + 1) * 512],
                                              wv[:, h, jcp, :, 0:65],
                                              pv[:, 2 * u:2 * u + 2,
                                                 q * 512:(q + 1) * 512],
                                              start=(jcp == 0),
                                              stop=(jcp == JC // 2 - 1),
                                              perf_mode=mybir.MatmulPerfMode.DoubleRow,
                                              skip_group_check=True)
                              else:
                                  for jl in range(G):
                                      jc = jcg * G + jl
                                      for q in range(W_I // 512):
                                          nc.tensor.matmul(
                                              acc[hh][:, q * 512:(q + 1) * 512],
                                              whaug[:, (jc * H + h) * 65:
                                                    (jc * H + h + 1) * 65],
                                              p_t[:, jl * W_I + q * 512:
                                                  jl * W_I + (q + 1) * 512],
                                              start=(jc == 0), stop=(jc == JC - 1),
                                              skip_group_check=True)
                      for hh in range(2):
                          h = hp * 2 + hh
                          nc.scalar.copy(
                              nd[h][:, ib * W_I:(ib + 1) * W_I], acc[hh])

          # ------------ phase D: normalize + head mean + transpose ------
          # Division via K=1 broadcast matmul: recipB[d, i] = ones[d]*recip[i]
          # lands in PSUM; DVE multiplies + accumulates heads per 512-chunk;
          # one transpose round on the combined [64, N] result.
          with tc.tile_pool(name="psD", bufs=4, space="PSUM") as psD, \
               tc.tile_pool(name="oc", bufs=3) as oc, \
               tc.tile_pool(name="rcp", bufs=1) as rcp:
              ones65 = rcp.tile([1, 65], f16, name="ones65")
              nc.vector.memset(ones65, 1.0)
              rrows = [rcp.tile([1, N], f16, name=f"rrow{h}", tag=f"rr{h}")
                       for h in range(H)]
              for h in range(H):
                  with nc.allow_low_precision(reason="softmax denom"):
                      nc.vector.reciprocal(rrows[h], nd[h][64:65, :])
              om = oc.tile([64, N], f32, name="om", bufs=1)
              for iq in range(N // 512):
                  o_prev = None
                  for h in range(H):
                      rb = psD.tile([64, 512], f32, tag="rb")
                      nc.tensor.matmul(
                          rb, ones65[0:1, 0:64],
                          rrows[h][:, iq * 512:(iq + 1) * 512],
                          start=True, stop=True, skip_group_check=True)
                      tgt_out = (om[:, iq * 512:(iq + 1) * 512]
                                 if h == H - 1 else None)
                      if h == 0:
                          o_t = oc.tile([64, 512], f32, tag="omw")
                          nc.vector.tensor_mul(
                              o_t, nd[h][0:64, iq * 512:(iq + 1) * 512], rb)
                          o_prev = o_t
                      else:
                          tmp = oc.tile([64, 512], f32, tag="omt")
                          nc.vector.tensor_mul(
                              tmp, nd[h][0:64, iq * 512:(iq + 1) * 512], rb)
                          dst = tgt_out if tgt_out is not None else oc.tile(
                              [64, 512], f32, tag="omw")
                          nc.vector.tensor_add(dst, o_prev, tmp)
                          o_prev = dst
              ob = oc.tile([128, (N // 128) * D], f32, name="ob", bufs=1)
              for blk in range(N // 128):
                  trp = psD.tile([128, 64], f32, tag="trp")
                  nc.tensor.transpose(
                      trp, om[:, blk * 128:(blk + 1) * 128], ident[0:64, 0:64])
                  nc.vector.tensor_copy(ob[:, blk * D:(blk + 1) * D], trp)
              nc.sync.dma_start(outd[:, :], ob)

    with tile.TileContext(nc) as tc:
        with tc.tile_pool(name="const", bufs=1) as const:
            wdt = mybir.dt.float8e4 if USE_FP8 else f16
            whaug = const.tile([128, JC * H * (80 if USE_FP8 else 65)], wdt)
            tgt16 = const.tile([128, JC * H], f16)
            srcB = const.tile([128, H * N], f16)
            ident = const.tile([65, 65], f32)
            make_identity(nc, ident)
            neg3 = const.tile([128, 1], f32)
            nc.vector.memset(neg3, -3.0)
            for _rep in range(reps):
                pipeline(tc, const, whaug, tgt16, srcB, ident, neg3)

    nc.compile()
    return nc


def _prep_inputs(x, adj_matrix_masked, W, attention):
    """Host-side shard/layout prep (slicing, transposes, weight packing)."""
    x = np.ascontiguousarray(x, dtype=np.float32)
    W = np.ascontiguousarray(W, dtype=np.float32)
    attention = np.ascontiguousarray(attention, dtype=np.float32)

    a_src = attention[:, :D, 0]          # [H, D]
    a_tgt = attention[:, D:, 0]          # [H, D]
    Wh_cols = W.reshape(C, H, D)
    w_src = np.einsum("chd,hd->ch", Wh_cols, a_src)   # [C, H]
    w_tgt = np.einsum("chd,hd->ch", Wh_cols, a_tgt)   # [C, H]

    waug = np.zeros((C, H * 65), np.float32)
    for h in range(H):
        waug[:, h * 65: h * 65 + 64] = 0.25 * Wh_cols[:, h, :]
        waug[:, h * 65 + 64] = w_tgt[:, h]
    waug = np.ascontiguousarray(waug.reshape(2, 128, H * 65))

    wsb = np.empty((H, 2, 128, 128), np.float32)
    for h in range(H):
        wsb[h] = np.repeat(w_src[:, h][:, None], 128, axis=1).reshape(2, 128, 128)

    in_maps = []
    for b in range(B):
        xTb = np.ascontiguousarray(x[b].T).reshape(2, 128, N)
        # mask -> [ib, jcg, p, (jl, i)] uint8 {0, 255}; value at
        # (ib, jcg, p, jl, i) = 255 * mask[b, 0, ib*2048 + i, (jcg*4+jl)*128+p]
        mb = adj_matrix_masked[b, 0]                       # [i, j] bool
        m = (mb.T.astype(np.uint8) * np.uint8(255))        # [j, i]
        m = m.reshape(8, 4, 128, 2, W_I)                   # jcg, jl, p, ib, i
        m = np.ascontiguousarray(m.transpose(3, 0, 2, 1, 4).reshape(
            2, 8, 128, G * W_I))
        in_maps.append(dict(xT=xTb, waug=waug, wsb=wsb, mprep=m))
    return in_maps


def _run(x, adj_matrix_masked, W, attention, reps=1):
    from concourse.bass_utils import run_bass_kernel_spmd

    key = f"nc{reps}"
    if key not in _cached:
        _cached[key] = _build(reps)
    nc = _cached[key]

    in_maps = _prep_inputs(x, adj_matrix_masked, W, attention)
    res = run_bass_kernel_spmd(nc, in_maps, core_ids=[0, 1])
    out = np.empty((B, N, D), np.float32)
    for b in range(B):
        ob = res.results[b]["out"]                         # [128, 32*64]
        out[b] = ob.reshape(128, N // 128, D).transpose(1, 0, 2).reshape(N, D)
    return out, res


def kernel(x, adj_matrix_masked, W, attention):
    out, _ = _run(x, adj_matrix_masked, W, attention)
    return out



# revision 2
# speedup vs baseline: 10.3457x; 10.3457x over previous
"""GAT conv layer (B=2, N=4096, C=256, H=4, D=64) on TRN2 NeuronCores.

Execution-environment reality (measured): each STATIC instruction in the
NEFF costs ~25-40us per execution of the (replicated) pipeline, regardless
of engine; instructions executed via For_i hardware loops pay only ~3.4us
per loop ITERATION plus near-architectural marginal per-element rates
(DVE ~190G f32 elem/s; both loop iterations and marginal work run in
parallel across cores).  The design therefore puts every hot phase inside
For_i hardware loops (static count ~100 vs ~1765 for the unrolled
baseline) and shards 8 ways: core = (batch b, target-quarter iq), each
core computing all 4 heads for 1024 target nodes against all 4096 sources.

Constraint: matmul ldweights (stationary operand, incl. transpose input)
cannot take register offsets -> inside loops, stationary data is staged
into fixed SBUF tiles with DVE copies (dynamic src), and matmuls use
static APs only.

Per-core pipeline (j = source chunk of 128, i = target, [j-part, i-free]
score layout so softmax needs no on-chip reduction):
  A: For_i jc in 32: stage x-chunk, psum = x_cc.T @ [0.25*W_h | w_tgt_h | 0]
     (both cc halves), copy to whaug fp16; then one strided memset sets the
     per-head 66th column to 1.0 (softmax-denominator ones column).
  B: For_i h in 4: stage w_src (broadcast columns), psum = wsb.T @ xTi
     -> srcB[h] (src scores broadcast across partitions).
  C: For_i jc in 32: DMA mask chunk; stage whaug chunk; per head:
     z = (srcB + tgt[j]) - 255*mask (STT), l = max(.2z, z) (STT),
     p = Exp(l) (ACT), acc_h[66, 1024] += [W|tgt|1].T @ p (2 matmuls,
     start/stop=False into pre-zeroed psum; row 65 accumulates the
     softmax denominator).  All 8 psum banks hold the 4 head accumulators.
  D: copy acc->sbuf, then For_i blk in 8: stage+transpose [66, 128] block
     per head, reciprocal of row 65, fused (num * recip + prev-head) STT
     chain -> out block [128 i, 64 d] (0.25 head-mean baked into W).
"""

import numpy as np

B, N, C, H, D = 2, 4096, 256, 4, 64
NEG = 0.2
JC = N // 128        # 32 source chunks
NQ = N // 4          # 1024 target nodes per core
NCORES = 8

_cached = {}


def _build(reps=1):
    import concourse.bacc as bacc
    import concourse.tile as tile
    from concourse import mybir
    from concourse.bass import ds
    from concourse.masks import make_identity

    f32 = mybir.dt.float32
    f16 = mybir.dt.float16
    u8 = mybir.dt.uint8
    Alu = mybir.AluOpType
    Exp = mybir.ActivationFunctionType.Exp

    nc = bacc.Bacc(None, target_bir_lowering=False, name="gat3")

    xTd = nc.dram_tensor("xT", [128, 2 * N], f16, kind="ExternalInput")
    xTid = nc.dram_tensor("xTi", [128, 2 * NQ], f16, kind="ExternalInput")
    waugd = nc.dram_tensor("waug", [128, 2 * 264], f16, kind="ExternalInput")
    wsbd = nc.dram_tensor("wsb", [128, H * 256], f16, kind="ExternalInput")
    mprepd = nc.dram_tensor("mprep", [128, JC * NQ], u8, kind="ExternalInput")
    outd = nc.dram_tensor("out", [128, (NQ // 128) * D], f32,
                          kind="ExternalOutput")

    def pipeline(tc, ident, z64):
        with tc.tile_pool(name="seq", bufs=1) as seq:
            xT = seq.tile([128, 2 * N], f16)
            xTi = seq.tile([128, 2 * NQ], f16)
            waug = seq.tile([128, 528], f16)
            wsb = seq.tile([128, H * 256], f16)
            whaug = seq.tile([128, JC * 264], f16)
            srcB = seq.tile([128, H * NQ], f16)
            nc.sync.dma_start(xT, xTd[:, :])
            nc.sync.dma_start(xTi, xTid[:, :])
            nc.sync.dma_start(waug, waugd[:, :])
            nc.sync.dma_start(wsb, wsbd[:, :])

            # ---------------- phase A: projection -------------------------
            with tc.tile_pool(name="pa", bufs=1) as pa, \
                 tc.tile_pool(name="psA", bufs=1, space="PSUM") as psA:
                xcur = pa.tile([128, 256], f16)
                psp = psA.tile([128, 264], f32)
                with tc.For_i(0, JC) as jc:
                    nc.vector.tensor_copy(xcur[:, 0:128],
                                          xT[:, ds(jc * 128, 128)])
                    nc.vector.tensor_copy(xcur[:, 128:256],
                                          xT[:, ds(N + jc * 128, 128)])
                    nc.tensor.matmul(psp, xcur[:, 0:128], waug[:, 0:264],
                                     start=True, stop=False)
                    nc.tensor.matmul(psp, xcur[:, 128:256], waug[:, 264:528],
                                     start=False, stop=True)
                    nc.scalar.copy(whaug[:, ds(jc * 264, 264)], psp)
                nc.vector.memset(
                    whaug.rearrange("p (g l) -> p g l", l=66)[:, :, 65:66], 1.0)

            # ---------------- phase B: srcB broadcast ---------------------
            with tc.tile_pool(name="pb", bufs=1) as pb, \
                 tc.tile_pool(name="psB", bufs=1, space="PSUM") as psB:
                wcur = pb.tile([128, 256], f16)
                pss = [psB.tile([128, 512], f32, name=f"pss{q}")
                       for q in range(2)]
                with tc.For_i(0, H) as h:
                    nc.vector.tensor_copy(wcur[:, 0:128],
                                          wsb[:, ds(h * 256, 128)])
                    nc.vector.tensor_copy(wcur[:, 128:256],
                                          wsb[:, ds(h * 256 + 128, 128)])
                    for q in range(2):
                        nc.tensor.matmul(
                            pss[q], wcur[:, 0:128],
                            xTi[:, q * 512:(q + 1) * 512],
                            start=True, stop=False)
                        nc.tensor.matmul(
                            pss[q], wcur[:, 128:256],
                            xTi[:, NQ + q * 512:NQ + (q + 1) * 512],
                            start=False, stop=True)
                        nc.scalar.copy(srcB[:, ds(h * NQ + q * 512, 512)],
                                       pss[q])

            # ---------------- phase C: scores + attention matmul ----------
            nd = [seq.tile([66, NQ], f32, name=f"nd{h}") for h in range(H)]
            with tc.tile_pool(name="mw", bufs=1) as mw, \
                 tc.tile_pool(name="pc", bufs=1) as pc, \
                 tc.tile_pool(name="psC", bufs=1, space="PSUM") as psC:
                acc = [psC.tile([66, NQ], f32, name=f"acc{h}")
                       for h in range(H)]
                for h in range(H):
                    nc.vector.memset(acc[h], 0.0)
                wc = pc.tile([128, 264], f16)
                m_t = mw.tile([128, NQ], u8)
                zts = [pc.tile([128, NQ], f16, name=f"z{h}") for h in range(H)]
                lts = [pc.tile([128, NQ], f16, name=f"l{h}") for h in range(H)]
                pts = [pc.tile([128, NQ], f16, name=f"p{h}") for h in range(H)]
                with tc.For_i(0, JC) as jc:
                    nc.sync.dma_start(m_t, mprepd[:, ds(jc * NQ, NQ)])
                    nc.vector.tensor_copy(wc, whaug[:, ds(jc * 264, 264)])
                    for h in range(H):
                        nc.vector.scalar_tensor_tensor(
                            out=zts[h], in0=srcB[:, h * NQ:(h + 1) * NQ],
                            scalar=wc[:, h * 66 + 64:h * 66 + 65],
                            in1=m_t, op0=Alu.add, op1=Alu.subtract)
                        nc.vector.scalar_tensor_tensor(
                            out=lts[h], in0=zts[h], scalar=NEG, in1=zts[h],
                            op0=Alu.mult, op1=Alu.max)
                        nc.scalar.activation(out=pts[h], in_=lts[h], func=Exp)
                        for q in range(2):
                            nc.tensor.matmul(
                                acc[h][:, q * 512:(q + 1) * 512],
                                wc[:, h * 66:(h + 1) * 66],
                                pts[h][:, q * 512:(q + 1) * 512],
                                start=False, stop=False,
                                skip_group_check=True)
                for h in range(H):
                    nc.scalar.copy(nd[h], acc[h])

            # ---------------- phase D: normalize + transpose + out --------
            with tc.tile_pool(name="pd", bufs=1) as pd, \
                 tc.tile_pool(name="psD", bufs=1, space="PSUM") as psD:
                ndc = [pd.tile([66, 128], f32, name=f"ndc{h}")
                       for h in range(H)]
                trp = [psD.tile([128, 66], f32, name=f"trp{h}")
                       for h in range(H)]
                rcp = [pd.tile([128, 1], f32, name=f"rcp{h}")
                       for h in range(H)]
                tacc = [pd.tile([128, 64], f32, name=f"ta{h}")
                        for h in range(H - 1)]
                ob = pd.tile([128, (NQ // 128) * D], f32)
                with tc.For_i(0, NQ // 128) as blk:
                    for h in range(H):
                        nc.vector.tensor_copy(ndc[h],
                                              nd[h][:, ds(blk * 128, 128)])
                        nc.tensor.transpose(trp[h], ndc[h], ident)
                        with nc.allow_low_precision(reason="softmax denom"):
                            nc.vector.reciprocal(rcp[h], trp[h][:, 65:66])
                    prev = z64
                    for h in range(H):
                        dstt = (ob[:, ds(blk * D, D)] if h == H - 1
                                else tacc[h])
                        nc.vector.scalar_tensor_tensor(
                            out=dstt, in0=trp[h][:, 0:64], scalar=rcp[h],
                            in1=prev, op0=Alu.mult, op1=Alu.add)
                        prev = dstt
                nc.sync.dma_start(outd[:, :], ob)

    with tile.TileContext(nc) as tc:
        with tc.tile_pool(name="const", bufs=1) as const:
            ident = const.tile([66, 66], f32)
            make_identity(nc, ident)
            z64 = const.tile([128, 64], f32)
            nc.vector.memset(z64, 0.0)
            for _rep in range(reps):
                pipeline(tc, ident, z64)

    nc.compile()
    return nc


def _prep_inputs(x, adj_matrix_masked, W, attention):
    """Host-side shard/layout prep (slicing, transposes, weight packing)."""
    x = np.ascontiguousarray(x, dtype=np.float32)
    W = np.ascontiguousarray(W, dtype=np.float32)
    attention = np.ascontiguousarray(attention, dtype=np.float32)

    a_src = attention[:, :D, 0]          # [H, D]
    a_tgt = attention[:, D:, 0]          # [H, D]
    Wh_cols = W.reshape(C, H, D)
    w_src = np.einsum("chd,hd->ch", Wh_cols, a_src)   # [C, H]
    w_tgt = np.einsum("chd,hd->ch", Wh_cols, a_tgt)   # [C, H]

    # waug: [C, H*66] = per head [0.25*W_h | w_tgt_h | 0], as [128, 2*264]
    waug = np.zeros((C, H * 66), np.float32)
    for h in range(H):
        waug[:, h * 66: h * 66 + 64] = 0.25 * Wh_cols[:, h, :]
        waug[:, h * 66 + 64] = w_tgt[:, h]
    waug16 = np.ascontiguousarray(
        waug.reshape(2, 128, 264).transpose(1, 0, 2).reshape(128, 528)
    ).astype(np.float16)

    # wsb: [128, h*256 + cc*128 + k] = w_src[cc*128+p, h]  (repeated over k)
    wsb16 = np.empty((128, H * 256), np.float16)
    for h in range(H):
        for cc in range(2):
            wsb16[:, h * 256 + cc * 128: h * 256 + (cc + 1) * 128] = \
                w_src[cc * 128:(cc + 1) * 128, h][:, None]

    in_maps = []
    for b in range(B):
        # xT16[p, cc*N + j] = x[b, j, cc*128+p]
        xT16 = np.ascontiguousarray(
            x[b].T.reshape(2, 128, N).transpose(1, 0, 2).reshape(128, 2 * N)
        ).astype(np.float16)
        # mask -> [j, i] uint8 {0,255}, blocked [jc, p, i]
        mT = (adj_matrix_masked[b, 0].T.astype(np.uint8) * np.uint8(255))
        mT = mT.reshape(JC, 128, N)
        for iq in range(4):
            sl = slice(iq * NQ, (iq + 1) * NQ)
            xTi16 = np.ascontiguousarray(
                np.stack([xT16[:, cc * N + iq * NQ: cc * N + (iq + 1) * NQ]
                          for cc in range(2)], axis=1).reshape(128, 2 * NQ))
            mprep = np.ascontiguousarray(
                mT[:, :, sl].transpose(1, 0, 2).reshape(128, JC * NQ))
            in_maps.append(dict(xT=xT16, xTi=xTi16, waug=waug16,
                                wsb=wsb16, mprep=mprep))
    return in_maps


def _run(x, adj_matrix_masked, W, attention, reps=1):
    from concourse.bass_utils import run_bass_kernel_spmd

    key = f"nc{reps}"
    if key not in _cached:
        _cached[key] = _build(reps)
    nc = _cached[key]

    in_maps = _prep_inputs(x, adj_matrix_masked, W, attention)
    res = run_bass_kernel_spmd(nc, in_maps, core_ids=list(range(NCORES)))
    out = np.empty((B, N, D), np.float32)
    for core in range(NCORES):
        b, iq = divmod(core, 4)
        ob = res.results[core]["out"]                  # [128, 8*64]
        out[b, iq * NQ:(iq + 1) * NQ] = (
            ob.reshape(128, NQ // 128, D).transpose(1, 0, 2).reshape(NQ, D))
    return out, res


def kernel(x, adj_matrix_masked, W, attention):
    out, _ = _run(x, adj_matrix_masked, W, attention)
    return out
